# revision 1
# baseline (speedup 1.0000x reference)
"""ComplexGRUCell forward on 8 Trainium2 NeuronCores.

Strategy (data-parallel, feat-major compute):
  - Shard batch B=65536 across 8 cores (8192 rows each).
  - Host-side: transpose x/h slices to [256, 8192] (feature-major) and
    pre-combine the 6 complex weight pairs into 8 stacked real matrices
    (one per pre-activation accumulator), transposed into matmul-stationary
    layout. Biases pre-combined per accumulator.
  - Device: for each 512-column batch tile, accumulate the 8 gate
    pre-activations with fp32r matmuls (features on partitions, batch on the
    free dim), apply sigmoid/tanh with per-partition biases on the scalar
    engine, do the complex-arithmetic elementwise work on the vector engine,
    and DMA the feature-major outputs back.
  - Host-side: transpose outputs back to [B, 256] and stack real/imag.

Self-contained: hardcodes B=65536, I=H=256, 8 cores.
"""

import numpy as np

import concourse.bass as bass  # noqa: F401
import concourse.mybir as mybir
import concourse.tile as tile
from concourse import bacc, bass_utils

F32 = mybir.dt.float32
F32R = mybir.dt.float32r
FP16 = mybir.dt.float16
AF = mybir.ActivationFunctionType

B_TOTAL = 65536
N_CORES = 8
B_LOC = B_TOTAL // N_CORES  # 8192
H = 256
NB = 512                    # batch columns per tile
N_TILES = B_LOC // NB       # 16
KC = H // 128               # 2 feature chunks (partition dim)

_GATE_ACCS = ["r_re", "r_im", "z_re", "z_im"]      # 8 k-chunks each
_CAND_ACCS = ["x3_re", "x3_im", "g3_re", "g3_im"]  # 4 k-chunks each
_STREAMS = ["xrT", "xiT", "hrT", "hiT"]

# Module-level knobs for the test harness (grading path leaves them alone).
TRACE = False
LAST_RESULT = None

_CACHED_NC = None


def _build_nc():
    nc = bacc.Bacc("TRN2", target_bir_lowering=False, debug=False,
                   num_devices=N_CORES)

    ins = {}
    for s in _STREAMS:
        ins[s] = nc.dram_tensor(s, (H, B_LOC), F32R, kind="ExternalInput")
    for g in _GATE_ACCS:
        ins["w_" + g] = nc.dram_tensor("w_" + g, (128, 8 * 256), F32R,
                                       kind="ExternalInput")
    for g in _CAND_ACCS:
        ins["w_" + g] = nc.dram_tensor("w_" + g, (128, 4 * 256), F32R,
                                       kind="ExternalInput")
    ins["biases"] = nc.dram_tensor("biases", (128, 16), F32,
                                   kind="ExternalInput")
    out_r = nc.dram_tensor("outT_r", (H, B_LOC), F32, kind="ExternalOutput")
    out_i = nc.dram_tensor("outT_i", (H, B_LOC), F32, kind="ExternalOutput")

    bias_col = {}
    for gi, g in enumerate(_GATE_ACCS + _CAND_ACCS):
        for mo in range(2):
            bias_col[(g, mo)] = gi * 2 + mo

    with tile.TileContext(nc) as tc:
        with (
            tc.tile_pool(name="wpool", bufs=1) as wpool,
            tc.tile_pool(name="mvpool", bufs=2) as mvpool,
            tc.tile_pool(name="spool", bufs=3) as spool,
            tc.tile_pool(name="tpool", bufs=2) as tpool,
            tc.tile_pool(name="opool", bufs=3) as opool,
            tc.tile_pool(name="psum", bufs=1, space="PSUM") as psum,
        ):
            # ---- one-time weight/bias loads -------------------------------
            # Ordered so only the r-gate weights gate the first matmuls:
            # r weights -> tile-0 data -> remaining weights.
            wt = {}
            wt_chunks = {}

            def load_w(g, n):
                t = wpool.tile([128, n * 256], F32R, name=f"wt_{g}",
                               tag=f"wt_{g}")
                nc.sync.dma_start(t[:], ins["w_" + g][:])
                wt[g] = t

            def load_w_chunked(g, n):
                for ki in range(n):
                    t = wpool.tile([128, 256], F32R, name=f"wt_{g}_{ki}",
                                   tag=f"wt_{g}_{ki}")
                    nc.sync.dma_start(
                        t[:], ins["w_" + g][:, ki * 256:(ki + 1) * 256])
                    wt_chunks[(g, ki)] = t

            def w_ap(g, ki, mo):
                if (g, ki) in wt_chunks:
                    return wt_chunks[(g, ki)][:, mo * 128:(mo + 1) * 128]
                return wt[g][:, ki * 256 + mo * 128:ki * 256 + (mo + 1) * 128]

            def load_mv(c0, nb, streams=(0, 1, 2, 3), mv=None):
                mv = {} if mv is None else mv
                for si in streams:
                    s = _STREAMS[si]
                    for k in range(KC):
                        m = mvpool.tile([128, nb], F32R, name=f"mv{si}{k}",
                                        tag=f"mv{si}{k}",
                                        padded_shape=[128, NB],
                                        bufs=3 if si >= 2 else 2)
                        nc.sync.dma_start(
                            m[:], ins[s][k * 128:(k + 1) * 128, c0:c0 + nb])
                        mv[(si, k)] = m
                return mv

            load_w_chunked("r_re", 8)
            mv0 = load_mv(0, NB, streams=(0, 1))
            load_w_chunked("r_im", 8)
            load_mv(0, NB, streams=(2, 3), mv=mv0)
            for g in ("z_re", "z_im"):
                load_w(g, 8)
            for g in _CAND_ACCS:
                load_w(g, 4)
            bt = wpool.tile([128, 16], F32, name="bias_t", tag="bias_t")
            nc.sync.dma_start(bt[:], ins["biases"][:])

            def bias_ap(g, mo):
                c = bias_col[(g, mo)]
                return bt[:, c:c + 1]

            # ---- per batch tile -------------------------------------------
            schedule = [(i * NB, NB) for i in range(N_TILES)]
            for t_idx, (c0, nb) in enumerate(schedule):
                mv = mv0 if t_idx == 0 else load_mv(c0, nb)

                def mk_pair(nm, tag):
                    return psum.tile([128, 2 * nb], F32, name=nm, tag=tag)

                p_r = [mk_pair(f"p_r{mo}", f"bankA{mo}") for mo in range(2)]
                p_z = [mk_pair(f"p_z{mo}", f"bankB{mo}") for mo in range(2)]

                def accum(pair, half, g, mo, streams):
                    n_mm = len(streams) * KC
                    j = 0
                    for si in streams:
                        for k in range(KC):
                            ki = (si - streams[0]) * KC + k
                            nc.tensor.matmul(
                                pair[:, half * nb:(half + 1) * nb],
                                w_ap(g, ki, mo), mv[(si, k)][:],
                                start=(j == 0), stop=(j == n_mm - 1))
                            j += 1

                ALL, XS, HS = [0, 1, 2, 3], [0, 1], [2, 3]
                for mo in range(2):
                    accum(p_r[mo], 0, "r_re", mo, ALL)
                    accum(p_r[mo], 1, "r_im", mo, ALL)
                    accum(p_z[mo], 0, "z_re", mo, ALL)
                    accum(p_z[mo], 1, "z_im", mo, ALL)

                p_g3 = [mk_pair(f"p_g{mo}", f"bankA{mo}") for mo in range(2)]
                p_x3 = [mk_pair(f"p_x{mo}", f"bankB{mo}") for mo in range(2)]
                for mo in range(2):
                    accum(p_g3[mo], 0, "g3_re", mo, HS)
                    accum(p_g3[mo], 1, "g3_im", mo, HS)
                    accum(p_x3[mo], 0, "x3_re", mo, XS)
                    accum(p_x3[mo], 1, "x3_im", mo, XS)

                # ---- elementwise epilogue per feature chunk ----------------
                for mo in range(2):
                    sr = spool.tile([128, 2 * nb], F32, name=f"sr{mo}", tag="sr", bufs=2)
                    sz = spool.tile([128, 2 * nb], F32, name=f"sz{mo}", tag="sz", bufs=2)
                    g3 = spool.tile([128, 2 * nb], F32, name=f"g3{mo}", tag="g3", bufs=2)
                    nn = spool.tile([128, 2 * nb], F32, name=f"nn{mo}", tag="nn")

                    nc.scalar.activation(sr[:, 0:nb], p_r[mo][:, 0:nb],
                                         AF.Sigmoid, bias=bias_ap("r_re", mo))
                    nc.scalar.activation(sr[:, nb:], p_r[mo][:, nb:],
                                         AF.Sigmoid, bias=bias_ap("r_im", mo))
                    nc.scalar.activation(sz[:, 0:nb], p_z[mo][:, 0:nb],
                                         AF.Sigmoid, bias=bias_ap("z_re", mo))
                    nc.scalar.activation(sz[:, nb:], p_z[mo][:, nb:],
                                         AF.Sigmoid, bias=bias_ap("z_im", mo))
                    nc.scalar.activation(g3[:, 0:nb], p_g3[mo][:, 0:nb],
                                         AF.Identity, bias=bias_ap("g3_re", mo))
                    nc.scalar.activation(g3[:, nb:], p_g3[mo][:, nb:],
                                         AF.Identity, bias=bias_ap("g3_im", mo))

                    # h3 = r * g3 (complex)
                    u = tpool.tile([128, 2 * nb], F32, name=f"u{mo}", tag="u")
                    v = tpool.tile([128, 2 * nb], F32, name=f"v{mo}", tag="v")
                    h3 = tpool.tile([128, 2 * nb], F32, name=f"h3{mo}", tag="h3")
                    ss = tpool.tile([128, 2 * nb], F32, name=f"ss{mo}", tag="ss")
                    nc.vector.tensor_mul(u[:], sr[:], g3[:])   # rr*g3r | ri*g3i
                    nc.vector.tensor_mul(v[:, 0:nb], sr[:, 0:nb], g3[:, nb:])
                    nc.vector.tensor_mul(v[:, nb:], sr[:, nb:], g3[:, 0:nb])
                    nc.vector.tensor_sub(h3[:, 0:nb], u[:, 0:nb], u[:, nb:])
                    nc.vector.tensor_add(h3[:, nb:], v[:, 0:nb], v[:, nb:])
                    # s = x3 + h3 ; n = tanh(s + bias_x3)
                    nc.vector.tensor_add(ss[:], p_x3[mo][:], h3[:])
                    nc.scalar.activation(nn[:, 0:nb], ss[:, 0:nb],
                                         AF.Tanh, bias=bias_ap("x3_re", mo))
                    nc.scalar.activation(nn[:, nb:], ss[:, nb:],
                                         AF.Tanh, bias=bias_ap("x3_im", mo))

                    # d = h - n ; out = n + z*d (complex)
                    d = tpool.tile([128, 2 * nb], F32, name=f"d{mo}", tag="d")
                    p = tpool.tile([128, 2 * nb], F32, name=f"p{mo}", tag="p")
                    q = tpool.tile([128, 2 * nb], F32, name=f"q{mo}", tag="q")
                    tm = tpool.tile([128, 2 * nb], F32, name=f"tm{mo}", tag="tm")
                    ot = opool.tile([128, 2 * nb], F32, name=f"ot{mo}", tag="ot")
                    nc.vector.tensor_sub(d[:, 0:nb],
                                         mv[(2, mo)][:].bitcast(F32), nn[:, 0:nb])
                    nc.vector.tensor_sub(d[:, nb:],
                                         mv[(3, mo)][:].bitcast(F32), nn[:, nb:])
                    nc.vector.tensor_mul(p[:], sz[:], d[:])    # zr*dr | zi*di
                    nc.vector.tensor_mul(q[:, 0:nb], sz[:, 0:nb], d[:, nb:])
                    nc.vector.tensor_mul(q[:, nb:], sz[:, nb:], d[:, 0:nb])
                    nc.vector.tensor_sub(tm[:, 0:nb], p[:, 0:nb], p[:, nb:])
                    nc.vector.tensor_add(tm[:, nb:], q[:, 0:nb], q[:, nb:])
                    nc.vector.tensor_add(ot[:], nn[:], tm[:])

                    nc.sync.dma_start(
                        out_r[mo * 128:(mo + 1) * 128, c0:c0 + nb], ot[:, 0:nb])
                    nc.sync.dma_start(
                        out_i[mo * 128:(mo + 1) * 128, c0:c0 + nb], ot[:, nb:])

    nc.compile()
    return nc


def _prep_weights(p):
    """Host-side weight/bias combination -> device layouts."""
    def stk(mats):  # list of [256,256] -> stationary layout [128, n*256]
        W = np.concatenate(mats, axis=1)          # [out=256, in_total]
        WT = np.ascontiguousarray(W.T)            # [in_total, 256]
        n = WT.shape[0] // 128
        return np.ascontiguousarray(
            WT.reshape(n, 128, 256).transpose(1, 0, 2).reshape(128, n * 256)
        ).astype(np.float32)

    w = {}
    w["w_r_re"] = stk([p["w1Wr"], -p["w1Wi"], p["r1Wr"], -p["r1Wi"]])
    w["w_r_im"] = stk([p["w1Wi"], p["w1Wr"], p["r1Wi"], p["r1Wr"]])
    w["w_z_re"] = stk([p["w2Wr"], -p["w2Wi"], p["r2Wr"], -p["r2Wi"]])
    w["w_z_im"] = stk([p["w2Wi"], p["w2Wr"], p["r2Wi"], p["r2Wr"]])
    w["w_x3_re"] = stk([p["w3Wr"], -p["w3Wi"]])
    w["w_x3_im"] = stk([p["w3Wi"], p["w3Wr"]])
    w["w_g3_re"] = stk([p["r3Wr"], -p["r3Wi"]])
    w["w_g3_im"] = stk([p["r3Wi"], p["r3Wr"]])

    bias = {
        "r_re": p["w1br"] - p["w1bi"] + p["r1br"] - p["r1bi"],
        "r_im": p["w1br"] + p["w1bi"] + p["r1br"] + p["r1bi"],
        "z_re": p["w2br"] - p["w2bi"] + p["r2br"] - p["r2bi"],
        "z_im": p["w2br"] + p["w2bi"] + p["r2br"] + p["r2bi"],
        "x3_re": p["w3br"] - p["w3bi"],
        "x3_im": p["w3br"] + p["w3bi"],
        "g3_re": p["r3br"] - p["r3bi"],
        "g3_im": p["r3br"] + p["r3bi"],
    }
    bcols = np.zeros((128, 16), dtype=np.float32)
    for gi, g in enumerate(_GATE_ACCS + _CAND_ACCS):
        for mo in range(2):
            bcols[:, gi * 2 + mo] = np.asarray(bias[g])[mo * 128:(mo + 1) * 128]
    w["biases"] = bcols
    return w


def kernel(**inputs):
    global _CACHED_NC, LAST_RESULT
    if _CACHED_NC is None:
        _CACHED_NC = _build_nc()
    nc = _CACHED_NC

    wmaps = _prep_weights(inputs)

    in_maps = []
    for c in range(N_CORES):
        sl = slice(c * B_LOC, (c + 1) * B_LOC)
        m = dict(wmaps)
        m["xrT"] = np.ascontiguousarray(np.asarray(inputs["xr"])[sl].T,
                                        dtype=np.float32)
        m["xiT"] = np.ascontiguousarray(np.asarray(inputs["xi"])[sl].T,
                                        dtype=np.float32)
        m["hrT"] = np.ascontiguousarray(np.asarray(inputs["hr"])[sl].T,
                                        dtype=np.float32)
        m["hiT"] = np.ascontiguousarray(np.asarray(inputs["hi"])[sl].T,
                                        dtype=np.float32)
        in_maps.append(m)

    kwargs = {}
    if TRACE:
        import sys, types
        try:
            from trn_agent_boot.trn_boot import _ntff_profile_via_ctypes
            mod = types.ModuleType("antenv.axon_hooks")
            mod._hook = _ntff_profile_via_ctypes('/opt/axon/libaxon_pjrt.so')
            mod.get_axon_ntff_profile_hook = lambda: mod._hook
            mod.set_axon_ntff_profile_hook = (
                lambda h: setattr(mod, "_hook", h))
            sys.modules["antenv.axon_hooks"] = mod
            kwargs["trace"] = True
        except Exception:
            pass

    res = bass_utils.run_bass_kernel_spmd(
        nc, in_maps, core_ids=list(range(N_CORES)), **kwargs)
    LAST_RESULT = res

    out = np.empty((2, B_TOTAL, H), dtype=np.float32)
    for c in range(N_CORES):
        sl = slice(c * B_LOC, (c + 1) * B_LOC)
        out[0, sl] = res.results[c]["outT_r"].T
        out[1, sl] = res.results[c]["outT_i"].T
    return out



# revision 4
# speedup vs baseline: 1.0174x; 1.0174x over previous
"""ComplexGRUCell forward on 8 Trainium2 NeuronCores — Gauss 3M + fp16.

Strategy (data-parallel, feat-major compute):
  - Shard batch B=65536 across 8 cores (8192 rows each).
  - Host-side: transpose x/h slices to [256, 8192] fp16 (feature-major),
    precompute s-streams sx = xr+xi, sh = hr+hi, and combine the complex
    weight pairs of the r/z gates and the x3 candidate path into Gauss
    3-multiplication form:
        P1 = Wr s,  P2 = -(Wr+Wi) xi,  P3 = (Wi-Wr) xr
        re = P1 + P2,  im = P1 + P3
    (3 real matmuls per complex linear instead of 4). The small g3 path
    stays in standard 4-matmul form so its pre-activation lands complete
    in PSUM and the scalar engine can apply the bias directly.
  - Device: PE accumulates in fp16 at 1 cyc/row; Gauss fixup adds
    (PSUM+PSUM -> SBUF fp16) run on DVE; sigmoid/tanh/identity with fused
    per-partition bias on the scalar engine; the complex elementwise tail
    runs on DVE in fp16 (2x mode) with the SBUF-only pieces (h3, d, ot)
    on the Pool engine (GPSIMD cannot touch PSUM).
  - Host-side: transpose fp16 outputs back to [B, 256] f32 and stack.

Self-contained: hardcodes B=65536, I=H=256, 8 cores.
"""

import numpy as np

import concourse.bass as bass  # noqa: F401
import concourse.mybir as mybir
import concourse.tile as tile
from concourse import bacc, bass_utils

F32 = mybir.dt.float32
F16 = mybir.dt.float16
AF = mybir.ActivationFunctionType
ALU = mybir.AluOpType

B_TOTAL = 65536
N_CORES = 8
B_LOC = B_TOTAL // N_CORES  # 8192
H = 256
NB = 512                    # batch columns per tile
KC = H // 128               # 2 feature chunks (partition dim)

_STREAMS = ["xrT", "xiT", "sxT", "hrT", "hiT", "shT"]
_SI = {s: i for i, s in enumerate(_STREAMS)}

# Gauss paths: (path) -> product -> streams consumed (KC chunks each).
_GAUSS_PATHS = {
    "r":  {"P1": ["sxT", "shT"], "P2": ["xiT", "hiT"], "P3": ["xrT", "hrT"]},
    "z":  {"P1": ["sxT", "shT"], "P2": ["xiT", "hiT"], "P3": ["xrT", "hrT"]},
    "x3": {"P1": ["sxT"], "P2": ["xiT"], "P3": ["xrT"]},
}
# g3 standard form: re consumes [hrT (R3r), hiT (-R3i)], im [hiT (R3r), hrT (R3i)]
_G3_STREAMS = {"re": ["hrT", "hiT"], "im": ["hiT", "hrT"]}

_ACCS = ["r_re", "r_im", "z_re", "z_im", "x3_re", "x3_im", "g3_re", "g3_im"]

# Module-level knobs for the test harness (grading path leaves them alone).
TRACE = False
LAST_RESULT = None
N_TILES = B_LOC // NB       # 16

_CACHED_NC = None


def _build_nc(n_tiles=N_TILES, num_devices=N_CORES):
    b_loc = n_tiles * NB
    nc = bacc.Bacc("TRN2", target_bir_lowering=False, debug=False,
                   num_devices=num_devices)

    ins = {}
    for s in _STREAMS:
        ins[s] = nc.dram_tensor(s, (H, b_loc), F16, kind="ExternalInput")
    for p, prods in _GAUSS_PATHS.items():
        for q, streams in prods.items():
            ncols = len(streams) * KC * 256
            ins[f"w_{p}_{q}"] = nc.dram_tensor(
                f"w_{p}_{q}", (128, ncols), F16, kind="ExternalInput")
    for half in ("re", "im"):
        ins[f"w_g3_{half}"] = nc.dram_tensor(
            f"w_g3_{half}", (128, 2 * KC * 256), F16, kind="ExternalInput")
    ins["biases"] = nc.dram_tensor("biases", (128, 16), F32,
                                   kind="ExternalInput")
    out_r = nc.dram_tensor("outT_r", (H, b_loc), F16, kind="ExternalOutput")
    out_i = nc.dram_tensor("outT_i", (H, b_loc), F16, kind="ExternalOutput")

    bias_col = {}
    for ai, a in enumerate(_ACCS):
        for mo in range(2):
            bias_col[(a, mo)] = ai * 2 + mo

    with tile.TileContext(nc) as tc:
        with (
            tc.tile_pool(name="wpool", bufs=1) as wpool,
            tc.tile_pool(name="mvpool", bufs=2) as mvpool,
            tc.tile_pool(name="spool", bufs=2) as spool,
            tc.tile_pool(name="opool", bufs=3) as opool,
            tc.tile_pool(name="psum", bufs=1, space="PSUM") as psum,
        ):
            # ---- one-time weight/bias loads -------------------------------
            wt = {}

            def load_w(key, ncols):
                t = wpool.tile([128, ncols], F16, name=f"wt_{key}",
                               tag=f"wt_{key}")
                nc.sync.dma_start(t[:], ins[f"w_{key}"][:])
                wt[key] = t

            def w_ap(key, ki, mo):
                t = wt[key]
                return t[:, ki * 256 + mo * 128:ki * 256 + (mo + 1) * 128]

            def load_mv(c0, streams=_STREAMS, mv=None):
                mv = {} if mv is None else mv
                for s in streams:
                    si = _SI[s]
                    nbufs = 3 if s in ("hrT", "hiT") else 2
                    for k in range(KC):
                        m = mvpool.tile([128, NB], F16, name=f"mv{si}{k}",
                                        tag=f"mv{si}{k}", bufs=nbufs)
                        nc.sync.dma_start(
                            m[:], ins[s][k * 128:(k + 1) * 128, c0:c0 + NB])
                        mv[(s, k)] = m
                return mv

            # r-gate weights first so tile-0 matmuls can start early.
            for q in ("P1", "P2", "P3"):
                load_w(f"r_{q}", 2 * KC * 256)
            mv0 = load_mv(0, streams=("sxT", "shT", "xiT", "hiT"))
            for q in ("P1", "P2", "P3"):
                load_w(f"z_{q}", 2 * KC * 256)
            load_mv(0, streams=("xrT", "hrT"), mv=mv0)
            for q in ("P1", "P2", "P3"):
                load_w(f"x3_{q}", KC * 256)
            for half in ("re", "im"):
                load_w(f"g3_{half}", 2 * KC * 256)
            bt = wpool.tile([128, 16], F32, name="bias_t", tag="bias_t")
            nc.sync.dma_start(bt[:], ins["biases"][:])

            def bias_ap(a, mo):
                c = bias_col[(a, mo)]
                return bt[:, c:c + 1]

            # ---- per batch tile -------------------------------------------
            for t_idx in range(n_tiles):
                c0 = t_idx * NB
                mv = mv0 if t_idx == 0 else load_mv(c0)

                for mo in range(2):
                    def p1_tile(p):
                        return psum.tile([128, NB], F32, name=f"p1_{p}{mo}",
                                         tag="p1", bufs=2)

                    def p23_tile(p):
                        return psum.tile([128, 2 * NB], F32,
                                         name=f"p23_{p}{mo}", tag="p23",
                                         bufs=2)

                    def accum(out_ap, wkey, streams, mo):
                        n_mm = len(streams) * KC
                        j = 0
                        for s in streams:
                            for k in range(KC):
                                ki = (j // KC) * KC + k
                                nc.tensor.matmul(
                                    out_ap, w_ap(wkey, ki, mo), mv[(s, k)][:],
                                    start=(j == 0), stop=(j == n_mm - 1))
                                j += 1

                    pp = {}
                    for p in ("r", "z", "x3"):
                        p1 = p1_tile(p)
                        p23 = p23_tile(p)
                        prods = _GAUSS_PATHS[p]
                        accum(p1[:], f"{p}_P1", prods["P1"], mo)
                        accum(p23[:, 0:NB], f"{p}_P2", prods["P2"], mo)
                        accum(p23[:, NB:], f"{p}_P3", prods["P3"], mo)
                        pp[p] = (p1, p23)
                    pg = psum.tile([128, 2 * NB], F32, name=f"pg{mo}",
                                   tag="pg", bufs=1)
                    accum(pg[:, 0:NB], "g3_re", _G3_STREAMS["re"], mo)
                    accum(pg[:, NB:], "g3_im", _G3_STREAMS["im"], mo)

                    # ---- epilogue for this (tile, mo) ----------------------
                    # Gauss fixups: [re|im] = P23 + [P1|P1]. The DVE can only
                    # read ONE input from PSUM, so P1 is first copied to SBUF
                    # fp16 by the scalar engine (which has slack).
                    def fixup(p, out_t):
                        p1, p23 = pp[p]
                        p1sb = spool.tile([128, NB], F16, name=f"p1sb_{p}{mo}",
                                          tag=f"p1sb_{p}")
                        nc.scalar.copy(p1sb[:], p1[:])
                        in0 = p23[:].rearrange("p (a b) -> p a b", a=2)
                        in1 = p1sb[:].unsqueeze(1).broadcast_to([128, 2, NB])
                        out_ap = out_t[:].rearrange("p (a b) -> p a b", a=2)
                        nc.vector.tensor_add(out_ap, in0, in1)

                    rpre = spool.tile([128, 2 * NB], F16, name=f"rpre{mo}",
                                      tag="rpre")
                    zpre = spool.tile([128, 2 * NB], F16, name=f"zpre{mo}",
                                      tag="zpre")
                    x3p = spool.tile([128, 2 * NB], F16, name=f"x3p{mo}",
                                     tag="x3p")
                    g3 = spool.tile([128, 2 * NB], F16, name=f"g3{mo}",
                                    tag="g3")
                    fixup("r", rpre)
                    fixup("z", zpre)
                    fixup("x3", x3p)
                    # g3 pre-act is complete in PSUM: bias via scalar engine.
                    nc.scalar.activation(g3[:, 0:NB], pg[:, 0:NB],
                                         AF.Identity,
                                         bias=bias_ap("g3_re", mo))
                    nc.scalar.activation(g3[:, NB:], pg[:, NB:],
                                         AF.Identity,
                                         bias=bias_ap("g3_im", mo))

                    sr = spool.tile([128, 2 * NB], F16, name=f"sr{mo}",
                                    tag="sr")
                    sz = spool.tile([128, 2 * NB], F16, name=f"sz{mo}",
                                    tag="sz")
                    nc.scalar.activation(sr[:, 0:NB], rpre[:, 0:NB],
                                         AF.Sigmoid, bias=bias_ap("r_re", mo))
                    nc.scalar.activation(sr[:, NB:], rpre[:, NB:],
                                         AF.Sigmoid, bias=bias_ap("r_im", mo))
                    nc.scalar.activation(sz[:, 0:NB], zpre[:, 0:NB],
                                         AF.Sigmoid, bias=bias_ap("z_re", mo))
                    nc.scalar.activation(sz[:, NB:], zpre[:, NB:],
                                         AF.Sigmoid, bias=bias_ap("z_im", mo))

                    # h3 = r * g3 (complex); ss = x3p + h3
                    u = spool.tile([128, 2 * NB], F16, name=f"u{mo}", tag="u")
                    v = spool.tile([128, 2 * NB], F16, name=f"v{mo}", tag="v")
                    h3 = spool.tile([128, 2 * NB], F16, name=f"h3{mo}",
                                    tag="h3")
                    ss = spool.tile([128, 2 * NB], F16, name=f"ss{mo}",
                                    tag="ss")
                    nn = spool.tile([128, 2 * NB], F16, name=f"nn{mo}",
                                    tag="nn")
                    nc.vector.tensor_mul(u[:], sr[:], g3[:])
                    nc.vector.tensor_mul(v[:, 0:NB], sr[:, 0:NB], g3[:, NB:])
                    nc.vector.tensor_mul(v[:, NB:], sr[:, NB:], g3[:, 0:NB])
                    nc.gpsimd.tensor_sub(h3[:, 0:NB], u[:, 0:NB], u[:, NB:])
                    nc.gpsimd.tensor_add(h3[:, NB:], v[:, 0:NB], v[:, NB:])
                    nc.vector.tensor_add(ss[:], x3p[:], h3[:])
                    nc.scalar.activation(nn[:, 0:NB], ss[:, 0:NB],
                                         AF.Tanh, bias=bias_ap("x3_re", mo))
                    nc.scalar.activation(nn[:, NB:], ss[:, NB:],
                                         AF.Tanh, bias=bias_ap("x3_im", mo))

                    # d = h - n ; out = n + z*d (complex)
                    d = spool.tile([128, 2 * NB], F16, name=f"d{mo}", tag="d")
                    p_ = spool.tile([128, 2 * NB], F16, name=f"p{mo}",
                                    tag="p")
                    q_ = spool.tile([128, 2 * NB], F16, name=f"q{mo}",
                                    tag="q")
                    tm = spool.tile([128, 2 * NB], F16, name=f"tm{mo}",
                                    tag="tm")
                    ot = opool.tile([128, 2 * NB], F16, name=f"ot{mo}",
                                    tag="ot")
                    nc.gpsimd.tensor_sub(d[:, 0:NB], mv[("hrT", mo)][:],
                                         nn[:, 0:NB])
                    nc.gpsimd.tensor_sub(d[:, NB:], mv[("hiT", mo)][:],
                                         nn[:, NB:])
                    nc.vector.tensor_mul(p_[:], sz[:], d[:])
                    nc.vector.tensor_mul(q_[:, 0:NB], sz[:, 0:NB], d[:, NB:])
                    nc.vector.tensor_mul(q_[:, NB:], sz[:, NB:], d[:, 0:NB])
                    nc.vector.tensor_sub(tm[:, 0:NB], p_[:, 0:NB],
                                         p_[:, NB:])
                    nc.vector.tensor_add(tm[:, NB:], q_[:, 0:NB],
                                         q_[:, NB:])
                    nc.gpsimd.tensor_add(ot[:], nn[:], tm[:])

                    nc.sync.dma_start(
                        out_r[mo * 128:(mo + 1) * 128, c0:c0 + NB],
                        ot[:, 0:NB])
                    nc.sync.dma_start(
                        out_i[mo * 128:(mo + 1) * 128, c0:c0 + NB],
                        ot[:, NB:])

    nc.compile()
    return nc


def _prep_weights(p):
    """Host-side weight/bias combination -> device layouts (fp16)."""
    def stk(mats):  # list of [256,256] -> stationary layout [128, n*256]
        W = np.concatenate(mats, axis=1)          # [out=256, in_total]
        WT = np.ascontiguousarray(W.T)            # [in_total, 256]
        n = WT.shape[0] // 128
        return np.ascontiguousarray(
            WT.reshape(n, 128, 256).transpose(1, 0, 2).reshape(128, n * 256)
        ).astype(np.float16)

    def gauss(Wr, Wi):
        return Wr, -(Wr + Wi), (Wi - Wr)

    w = {}
    for path, (wx, wh) in (("r", ("w1", "r1")), ("z", ("w2", "r2"))):
        aW = gauss(p[wx + "Wr"], p[wx + "Wi"])   # x-side linear
        aR = gauss(p[wh + "Wr"], p[wh + "Wi"])   # h-side linear
        for qi, q in enumerate(("P1", "P2", "P3")):
            w[f"w_{path}_{q}"] = stk([aW[qi], aR[qi]])
    aX = gauss(p["w3Wr"], p["w3Wi"])
    for qi, q in enumerate(("P1", "P2", "P3")):
        w[f"w_x3_{q}"] = stk([aX[qi]])
    w["w_g3_re"] = stk([p["r3Wr"], -p["r3Wi"]])
    w["w_g3_im"] = stk([p["r3Wr"], p["r3Wi"]])

    bias = {
        "r_re": p["w1br"] - p["w1bi"] + p["r1br"] - p["r1bi"],
        "r_im": p["w1br"] + p["w1bi"] + p["r1br"] + p["r1bi"],
        "z_re": p["w2br"] - p["w2bi"] + p["r2br"] - p["r2bi"],
        "z_im": p["w2br"] + p["w2bi"] + p["r2br"] + p["r2bi"],
        "x3_re": p["w3br"] - p["w3bi"],
        "x3_im": p["w3br"] + p["w3bi"],
        "g3_re": p["r3br"] - p["r3bi"],
        "g3_im": p["r3br"] + p["r3bi"],
    }
    bcols = np.zeros((128, 16), dtype=np.float32)
    for ai, a in enumerate(_ACCS):
        for mo in range(2):
            bcols[:, ai * 2 + mo] = np.asarray(bias[a])[mo * 128:(mo + 1) * 128]
    w["biases"] = bcols
    return w


def kernel(**inputs):
    global _CACHED_NC, LAST_RESULT
    if _CACHED_NC is None:
        _CACHED_NC = _build_nc()
    nc = _CACHED_NC

    wmaps = _prep_weights(inputs)

    xr = np.asarray(inputs["xr"])
    xi = np.asarray(inputs["xi"])
    hr = np.asarray(inputs["hr"])
    hi = np.asarray(inputs["hi"])

    in_maps = []
    for c in range(N_CORES):
        sl = slice(c * B_LOC, (c + 1) * B_LOC)
        m = dict(wmaps)
        m["xrT"] = np.ascontiguousarray(xr[sl].T).astype(np.float16)
        m["xiT"] = np.ascontiguousarray(xi[sl].T).astype(np.float16)
        m["sxT"] = np.ascontiguousarray((xr[sl] + xi[sl]).T).astype(np.float16)
        m["hrT"] = np.ascontiguousarray(hr[sl].T).astype(np.float16)
        m["hiT"] = np.ascontiguousarray(hi[sl].T).astype(np.float16)
        m["shT"] = np.ascontiguousarray((hr[sl] + hi[sl]).T).astype(np.float16)
        in_maps.append(m)

    kwargs = {}
    if TRACE:
        import sys, types
        try:
            from trn_agent_boot.trn_boot import _ntff_profile_via_ctypes
            mod = types.ModuleType("antenv.axon_hooks")
            mod._hook = _ntff_profile_via_ctypes('/opt/axon/libaxon_pjrt.so')
            mod.get_axon_ntff_profile_hook = lambda: mod._hook
            mod.set_axon_ntff_profile_hook = (
                lambda h: setattr(mod, "_hook", h))
            sys.modules["antenv.axon_hooks"] = mod
            kwargs["trace"] = True
        except Exception:
            pass

    res = bass_utils.run_bass_kernel_spmd(
        nc, in_maps, core_ids=list(range(N_CORES)), **kwargs)
    LAST_RESULT = res

    out = np.empty((2, B_TOTAL, H), dtype=np.float32)
    for c in range(N_CORES):
        sl = slice(c * B_LOC, (c + 1) * B_LOC)
        out[0, sl] = res.results[c]["outT_r"].astype(np.float32).T
        out[1, sl] = res.results[c]["outT_i"].astype(np.float32).T
    return out
